# revision 23
# baseline (speedup 1.0000x reference)
"""BitNet-style MLP (rmsnorm -> act-quant -> ternary fc1 -> StarReLU ->
rmsnorm -> act-quant -> ternary fc2) on 8 Trainium2 NeuronCores.

Sharding: data-parallel over tokens (8192 tokens -> 1024/core). Weights are
replicated; the global mean(|w|) statistic is computed cooperatively (each
core reduces 1/8 of each weight, then a tiny AllReduce), after which every
core quantizes the full weight set to ternary fp8 on the fly.

All quantization math runs on device. Host-side work is limited to layout
(reshape/transpose/slicing) and the final gather.

Key numeric trick: the fake-quantized matmuls are integer-exact, so both
matmuls run on the PE in bf16 (activations = round(x*scale), ints in
[-128,127], exact in bf16) x fp8 (ternary weights in {-1,0,1}), with exact
f32 accumulation in PSUM; per-row dequant scales are applied at PSUM
eviction. Rounding uses the (x + 1.5*2^23) - 1.5*2^23 magic-number trick,
which matches jnp.round's round-half-to-even exactly.
"""

import numpy as np
from contextlib import ExitStack

import concourse.bass as bass
import concourse.tile as tile
from concourse import bacc, mybir
from concourse.bass_utils import run_bass_kernel_spmd

AO = mybir.AluOpType
AF = mybir.ActivationFunctionType
F32 = mybir.dt.float32
BF16 = mybir.dt.bfloat16
FP8 = mybir.dt.float8e4

MAGIC = 1.5 * 2 ** 23  # RNE integer rounding for |v| <= 2^22
N_CORES = 8
B, S, D, H = 4, 2048, 1024, 4096
TOK_TOTAL = B * S                 # 8192
TOK = TOK_TOTAL // N_CORES        # 1024 tokens per core
NT = TOK // 128                   # 8 token tiles per core
DK = D // 128                     # 8 k-tiles for fc1
HK = H // 128                     # 32 k-tiles for fc2
HN = H // 512                     # 8 n-chunks for fc1
DN = D // 512                     # 2 n-chunks for fc2
EPS_NORM = 1e-08
EPS_Q = 1e-05

DEFAULT_CFG = dict(
    big_bufs=4,
    mid_bufs=3,
    stats_bufs=6,
    junk_step0=True,
    junk_bufs=2,
    hq_bufs=2,
    xq_bufs=2,
    xt_bufs=2,
    ht_bufs=2,
    fc1_ps_bufs=3,
    fc2_ps_bufs=1,
    # The Pool/GpSimd engine is ~10x slower than modeled on real TRN2 for
    # bulk streams: keep every bulk element-wise op on DVE/ACT instead.
    w_ts3_engine="vector",   # engine for final clip pass of weight quant
    quant_sub_engine="act",  # engine for the magic-number subtract
    x_sq_engine="scalar",    # engine for sum(x^2)
)


def _step0_free_ap(t_ap: bass.AP, count: int) -> bass.AP:
    """AP that 'writes' `count` elements per partition all landing on the
    same address (free step 0) - a bit bucket for ops whose only useful
    output is accum_out."""
    part = t_ap.ap[0]
    return bass.AP(t_ap.tensor, t_ap.offset, [[part[0], part[1]], [0, count]])


def build(s_act: float, b_act: float, use_n1: bool, use_n2: bool, cfg=None):
    """Build the per-core Bass module (SPMD: same module on all 8 cores)."""
    cfg = dict(DEFAULT_CFG, **(cfg or {}))
    general = use_n1 or use_n2 or (b_act != 0.0) or (s_act < 0.0)
    fast_act = (b_act == 0.0) and (s_act >= 0.0)

    nc = bacc.Bacc(None, target_bir_lowering=False)

    x_d = nc.dram_tensor("x", [TOK, D], F32, kind="ExternalInput")
    w1t_d = nc.dram_tensor("w1t", [D, H], F32, kind="ExternalInput")
    w2t_d = nc.dram_tensor("w2t", [H, D], F32, kind="ExternalInput")
    w1s_d = nc.dram_tensor("w1s", [D // N_CORES, H], F32, kind="ExternalInput")
    w2s_d = nc.dram_tensor("w2s", [H // N_CORES, D], F32, kind="ExternalInput")
    if use_n1:
        n1_d = nc.dram_tensor("n1", [1, D], F32, kind="ExternalInput")
    if use_n2:
        n2_d = nc.dram_tensor("n2", [1, H], F32, kind="ExternalInput")
    y_d = nc.dram_tensor("y", [TOK, D], F32, kind="ExternalOutput")

    with tile.TileContext(nc) as tc:
        with ExitStack() as ctx:
            const = ctx.enter_context(tc.tile_pool(name="const", bufs=1))
            stats = ctx.enter_context(tc.tile_pool(name="stats", bufs=cfg["stats_bufs"]))
            wq_pool = ctx.enter_context(tc.tile_pool(name="wq", bufs=1))
            big = ctx.enter_context(tc.tile_pool(name="big", bufs=cfg["big_bufs"]))
            mid = ctx.enter_context(tc.tile_pool(name="mid", bufs=cfg["mid_bufs"]))
            w2st_pool = ctx.enter_context(tc.tile_pool(name="w2st", bufs=3))
            xq_pool = ctx.enter_context(tc.tile_pool(name="xq", bufs=cfg["xq_bufs"]))
            xt_pool = ctx.enter_context(tc.tile_pool(name="xT", bufs=cfg["xt_bufs"]))
            hq_pool = ctx.enter_context(tc.tile_pool(name="hq", bufs=cfg["hq_bufs"]))
            ht_pool = ctx.enter_context(tc.tile_pool(name="hT", bufs=cfg["ht_bufs"]))
            junk_pool = ctx.enter_context(tc.tile_pool(name="junk", bufs=cfg["junk_bufs"]))
            fc1_ps = ctx.enter_context(tc.tile_pool(name="fc1ps", bufs=cfg["fc1_ps_bufs"], space="PSUM"))
            fc2_ps = ctx.enter_context(tc.tile_pool(name="fc2ps", bufs=cfg["fc2_ps_bufs"], space="PSUM"))
            dram = ctx.enter_context(tc.tile_pool(name="dram", bufs=1, space="DRAM"))

            ones_col = const.tile([128, 1], F32)
            nc.vector.memset(ones_col[:], 1.0)
            magic_b = const.tile([128, 1], F32)
            nc.vector.memset(magic_b[:], MAGIC)
            negmagic_b = const.tile([128, 1], F32)
            nc.vector.memset(negmagic_b[:], -MAGIC)
            eps_b = const.tile([128, 1], F32)
            nc.vector.memset(eps_b[:], EPS_NORM)

            qsub_eng = cfg["quant_sub_engine"]

            def emit_quant_sub(out_ap, in_ap):
                """(v + MAGIC) -> v as bf16 ints; engine-selectable."""
                if qsub_eng == "act":
                    nc.scalar.activation(out_ap, in_ap, AF.Identity, bias=negmagic_b[:])
                elif qsub_eng == "vector":
                    nc.vector.tensor_scalar(out_ap, in_ap, MAGIC, None, AO.subtract)
                else:
                    nc.gpsimd.tensor_scalar(out_ap, in_ap, MAGIC, None, AO.subtract)

            # wstats columns: 0=mw1, 1=mw2, 2=s1, 3=s2
            wstats = stats.tile([128, 4], F32, tag="wstats")
            mw1_ap, mw2_ap = wstats[:, 0:1], wstats[:, 1:2]
            s1_ap, s2_ap = wstats[:, 2:3], wstats[:, 3:4]

            if cfg["junk_step0"]:
                junk_small = junk_pool.tile([128, 8], BF16, tag="junk")

                def junk_ap(n):
                    return _step0_free_ap(junk_small[:, 0:1], n)
            else:
                junk_big = junk_pool.tile([128, H], BF16, tag="junk")

                def junk_ap(n):
                    return junk_big[:, 0:n]

            if use_n1:
                n1_sb = const.tile([128, D], F32)
                a0 = n1_d[:]
                nc.sync.dma_start(n1_sb[:], bass.AP(a0.tensor, a0.offset, [[0, 128], [1, D]]))
            if use_n2:
                n2_sb = const.tile([128, H], F32)
                a0 = n2_d[:]
                nc.sync.dma_start(n2_sb[:], bass.AP(a0.tensor, a0.offset, [[0, 128], [1, H]]))

            # -------- W phase: global mean(|w|) via ONE tiny AllReduce --------
            # split in two so the slice loads + local reduction (part1) can be
            # emitted at the very top of the program (they gate everything the
            # PE does), while the collective itself is emitted after X(0)/X(1)
            # (Tile serializes DMA transposes against earlier collectives).
            def weight_means_part1():
                # each slice is loaded in two halves so the |w| reduction
                # overlaps the DMA; the four partial sums ride through one
                # [4,1] PE column-sum and a 16-byte AllReduce.
                ar_in = dram.tile([4, 1], F32, tag="ari")
                ar_out = dram.tile([4, 1], F32, tag="aro")
                parts = stats.tile([128, 4], F32, tag="wpart")
                # column order (w1h0, w2h0, w1h1, w2h1) so part2 can pair the
                # half-sums with two contiguous [128,2] slices.
                for half in range(2):
                    for tagn, slice_dram in ((0, w1s_d), (1, w2s_d)):
                        rows, fdim = slice_dram.shape
                        st_t = big.tile([128, rows // 128, fdim // 2],
                                        F32, tag="big", name=f"wst{tagn}{half}")
                        src = slice_dram[:, (fdim // 2) * half:(fdim // 2) * (half + 1)]
                        nc.sync.dma_start(st_t[:], src.rearrange("(c p) f -> p c f", p=128))
                        col = 2 * half + tagn
                        nc.vector.tensor_reduce(parts[:, col:col + 1], st_t[:],
                                                mybir.AxisListType.XY, AO.add,
                                                apply_absolute_value=True)
                # cross-partition sums: [4,1] = parts.T @ ones
                pss = fc2_ps.tile([4, 1], F32, tag="fc2ps")
                nc.tensor.matmul(pss[:], parts[:], ones_col[:], start=True, stop=True)
                sb1 = stats.tile([4, 1], F32, tag="wsum")
                nc.scalar.activation(sb1[:], pss[:], AF.Copy)
                nc.sync.dma_start(ar_in[:], sb1[:])
                return ar_in, ar_out

            def weight_means_part2(ar_in, ar_out):
                rg = ([[c] for c in range(N_CORES)] if cfg.get("rg_single")
                      else [list(range(N_CORES))])
                nc.gpsimd.collective_compute(
                    "AllReduce", AO.add,
                    replica_groups=rg,
                    ins=[ar_in[:].opt()], outs=[ar_out[:].opt()],
                )
                bc = stats.tile([128, 4], F32, tag="wbc")
                a0 = ar_out[:]
                nc.sync.dma_start(bc[:], bass.AP(a0.tensor, a0.offset, [[0, 128], [1, 4]]))
                # pair the half-sums, then scale/clamp/reciprocal as before
                paired = stats.tile([128, 2], F32, tag="wpaired")
                nc.vector.tensor_tensor(paired[:], bc[:, 0:2], bc[:, 2:4], AO.add)
                nc.vector.tensor_scalar(wstats[:, 0:2], paired[:], 1.0 / (D * H), EPS_Q,
                                        AO.mult, AO.max)
                nc.vector.reciprocal(wstats[:, 2:4], wstats[:, 0:2])

            # ---------------- weight quantization (full, streamed) -----------
            w1q = wq_pool.tile([128, DK, H], FP8, tag="w1q")
            w2q = wq_pool.tile([128, HK, D], FP8, tag="w2q")
            ts3_eng = nc.gpsimd if cfg["w_ts3_engine"] == "gpsimd" else nc.vector

            def prefetch_w1(k):
                wt = big.tile([128, H], F32, tag="big")
                nc.sync.dma_start(wt[:], w1t_d[128 * k:128 * (k + 1), :])
                return wt

            def emit_w1_quant(k, wt=None):
                if wt is None:
                    wt = prefetch_w1(k)
                nc.scalar.activation(wt[:], wt[:], AF.Identity, scale=s1_ap, bias=magic_b[:])
                nc.vector.tensor_scalar(wt[:], wt[:], MAGIC, 1.0, AO.subtract, AO.min)
                ts3_eng.tensor_scalar(w1q[:, k, :], wt[:], -1.0, None, AO.max)

            W2B = 1   # w2t chunks per staged load

            def emit_w2_quant(j):
                wt = w2st_pool.tile([128, D], F32, tag="w2stage")
                nc.sync.dma_start(wt[:], w2t_d[128 * j:128 * (j + 1), :])
                nc.scalar.activation(wt[:], wt[:], AF.Identity, scale=s2_ap, bias=magic_b[:])
                nc.vector.tensor_scalar(wt[:], wt[:], MAGIC, 1.0, AO.subtract, AO.min)
                ts3_eng.tensor_scalar(w2q[:, j, :], wt[:], -1.0, None, AO.max)

            # ---------------- main token-tile pipeline ------------------------
            tiles = [dict() for _ in range(NT)]

            def rsqrt_newton(st, u, q, rs0):
                """rs = 1/sqrt(u), Newton-refined to ~1ulp (ACT Sqrt alone is
                only ~8ulp and rounding-boundary flips are costly here)."""
                nA, nB, rsr = st[:, 12:13], st[:, 13:14], st[:, 14:15]
                nc.scalar.activation(q, u, AF.Sqrt)
                nc.vector.reciprocal(rs0, q)
                nc.vector.tensor_tensor(nA, rs0, rs0, AO.mult)
                nc.vector.tensor_tensor(nB, nA, u, AO.mult)
                nc.vector.tensor_scalar(nA, nB, -0.5, 1.5, AO.mult, AO.add)
                nc.vector.tensor_tensor(rsr, rs0, nA, AO.mult)
                return rsr

            def emit_X(i):
                t = tiles[i]
                xt = mid.tile([128, D], F32, tag="x")
                nc.sync.dma_start(xt[:], x_d[128 * i:128 * (i + 1), :])
                st = stats.tile([128, 16], F32, tag="st")
                ssum, m1 = st[:, 0:1], st[:, 1:2]
                u, q, rs = st[:, 2:3], st[:, 3:4], st[:, 4:5]
                amax, ia = st[:, 5:6], st[:, 6:7]
                c1, d1 = st[:, 7:8], st[:, 8:9]
                nc.scalar.activation(junk_ap(D), xt[:], AF.Square, accum_out=ssum)
                if use_n1:
                    xw = mid.tile([128, D], F32, tag="xw")
                    nc.vector.tensor_tensor(xw[:], xt[:], n1_sb[:], AO.mult)
                    quant_src = xw
                else:
                    quant_src = xt
                nc.vector.tensor_reduce(m1, quant_src[:], mybir.AxisListType.X,
                                        AO.max, apply_absolute_value=True)
                nc.vector.tensor_scalar(u, ssum, 1.0 / D, EPS_NORM, AO.mult, AO.add)
                rs = rsqrt_newton(st, u, q, rs)
                nc.vector.tensor_scalar(amax, m1, rs, EPS_Q, AO.mult, AO.max)
                nc.vector.reciprocal(ia, amax)
                nc.vector.tensor_scalar(c1, ia, rs, 127.0, AO.mult, AO.mult)
                nc.vector.tensor_scalar(quant_src[:], quant_src[:], c1, MAGIC, AO.mult, AO.add)
                xq = xq_pool.tile([128, D], BF16, tag="xq")
                emit_quant_sub(xq[:], quant_src[:])
                xT = xt_pool.tile([128, DK, 128], BF16, tag="xT")
                # one multi-tile xbar transpose: xT[p,k,t] = xq[t, 128k+p]
                nc.sync.dma_start_transpose(xT[:], xq[:])
                t["xT"], t["st"], t["d1"] = xT, st, d1

            def emit_FC1(i):
                t = tiles[i]
                xT, st = t["xT"], t["st"]
                amax, d1 = st[:, 5:6], st[:, 8:9]
                # fast path folds sqrt(act_scale) into the fc1 dequant scale so
                # h' = (relu(h)*sqrt(s))^2 = s*relu(h)^2 with no extra pass.
                # computed here (not in emit_X) because mw1 is written by the
                # weight-mean collective, which is emitted after X(0)/X(1).
                d1_k = (np.sqrt(s_act) if fast_act else 1.0) / 127.0
                nc.vector.tensor_scalar(d1, amax, mw1_ap, d1_k, AO.mult, AO.mult)
                r = big.tile([128, H], F32, tag="big")
                for half in range(2):
                    # two 2-bank psum tiles per half; one wide ACT evict each
                    pss = [fc1_ps.tile([128, 1024], F32, tag="fc1ps", name=f"fc1ps{j}") for j in range(HN // 4)]
                    for k in range(DK):
                        for j in range(HN // 2):
                            n = half * (HN // 2) + j
                            nc.tensor.matmul(
                                pss[j // 2][:, 512 * (j % 2):512 * (j % 2 + 1)],
                                xT[:, k, :], w1q[:, k, 512 * n:512 * (n + 1)],
                                start=(k == 0), stop=(k == DK - 1))
                    for j in range(HN // 4):
                        n0 = half * (HN // 2) + 2 * j
                        nc.scalar.activation(r[:, 512 * n0:512 * (n0 + 2)], pss[j][:],
                                             AF.Relu, scale=d1)
                t["r"] = r

            def emit_H(i):
                # processed in column halves so the second half's stats work
                # overlaps the first half's, and FC2 can start on the first
                # half's transposed tiles while the second half finishes.
                t = tiles[i]
                r, st = t["r"], t["st"]
                HH = H // 2
                halves = (slice(0, HH), slice(HH, H))
                hsum, hmax = st[:, 9:10], st[:, 10:11]
                hsum_p = [st[:, 9:10], st[:, 15:16]]
                hmax_p = [st[:, 10:11], st[:, 11:12]]
                u2, q2, rs2 = st[:, 2:3], st[:, 3:4], st[:, 4:5]
                amax2, ia2 = st[:, 5:6], st[:, 6:7]
                c2, d2 = st[:, 7:8], st[:, 1:2]
                hp = big.tile([128, H], F32, tag="big")
                for j, sl in enumerate(halves):
                    nc.vector.tensor_tensor(hp[:, sl], r[:, sl], r[:, sl], AO.mult)
                    if not fast_act:
                        nc.vector.tensor_scalar(hp[:, sl], hp[:, sl], s_act, b_act,
                                                AO.mult, AO.add)
                    # v2 = mean(h'^2) uses h' BEFORE the norm2_w column scale
                    nc.scalar.activation(junk_ap(HH), hp[:, sl], AF.Square,
                                         accum_out=hsum_p[j])
                if use_n2:
                    hpw = big.tile([128, H], F32, tag="big")
                    nc.vector.tensor_tensor(hpw[:], hp[:], n2_sb[:], AO.mult)
                    hp = hpw
                for j, sl in enumerate(halves):
                    nc.vector.tensor_reduce(
                        hmax_p[j], hp[:, sl], mybir.AxisListType.X, AO.max,
                        apply_absolute_value=True if general else None)
                nc.vector.tensor_tensor(hsum, hsum_p[0], hsum_p[1], AO.add)
                nc.vector.tensor_tensor(hmax, hmax_p[0], hmax_p[1], AO.max)
                nc.vector.tensor_scalar(u2, hsum, 1.0 / H, EPS_NORM, AO.mult, AO.add)
                rs2 = rsqrt_newton(st, u2, q2, rs2)
                nc.vector.tensor_scalar(amax2, hmax, rs2, EPS_Q, AO.mult, AO.max)
                nc.vector.reciprocal(ia2, amax2)
                nc.vector.tensor_scalar(c2, ia2, rs2, 127.0, AO.mult, AO.mult)
                nc.vector.tensor_scalar(d2, amax2, mw2_ap, 1.0 / 127.0, AO.mult, AO.mult)
                hq = hq_pool.tile([128, H], BF16, tag="hq")
                hT = ht_pool.tile([128, HK, 128], BF16, tag="hT")
                for j, sl in enumerate(halves):
                    nc.vector.tensor_scalar(hp[:, sl], hp[:, sl], c2, MAGIC,
                                            AO.mult, AO.add)
                    emit_quant_sub(hq[:, sl], hp[:, sl])
                    nc.sync.dma_start_transpose(
                        hT[:, j * (HK // 2):(j + 1) * (HK // 2), :], hq[:, sl])
                t["hT"], t["d2"] = hT, d2

            def emit_FC2(i):
                t = tiles[i]
                hT, d2 = t["hT"], t["d2"]
                y_sb = mid.tile([128, D], F32, tag="y")
                ps2 = fc2_ps.tile([128, 1024], F32, tag="fc2ps")
                for n in range(DN):
                    for k in range(HK):
                        nc.tensor.matmul(
                            ps2[:, 512 * n:512 * (n + 1)], hT[:, k, :],
                            w2q[:, k, 512 * n:512 * (n + 1)],
                            start=(k == 0), stop=(k == HK - 1))
                nc.scalar.activation(y_sb[:], ps2[:], AF.Copy, scale=d2)
                nc.sync.dma_start(y_d[128 * i:128 * (i + 1), :], y_sb[:])

            # software-pipelined emission so the PE stream is
            # FC1(0) FC1(1) FC2(0) FC1(2) FC2(1) ... FC1(7) FC2(6) FC2(7).
            # x-stage for tiles 0/1 is emitted before the weight-quant ACT ops
            # so tile 0's critical path isn't queued behind them; w2 quant is
            # emitted after FC1(0) (it is only needed by FC2(0), much later).
            # Emission order notes:
            #  - X(0)/X(1) go first so their DMA transposes precede the
            #    collective in program order (Tile serializes transposes
            #    against earlier collectives) and tile 0's critical path
            #    isn't queued behind weight-quant ACT/DVE ops.
            #  - w2 quant is emitted after FC1(0): it is only needed by
            #    FC2(0), much later.
            NTL = cfg.get("nt") or NT
            # cfg["repeat"] re-emits the whole program N times inside one
            # NEFF; used by the timing harness to measure marginal
            # per-execution device time with dispatch/RTT overheads cancelled.
            for _rep in range(cfg.get("repeat", 1)):
                for t in tiles:
                    t.clear()
                ar_in, ar_out = weight_means_part1()
                w1_pre = {k: prefetch_w1(k) for k in range(2)}
                emit_X(0)
                if NTL > 1:
                    emit_X(1)
                weight_means_part2(ar_in, ar_out)
                for k in range(DK):
                    emit_w1_quant(k, w1_pre.get(k))
                emit_FC1(0)
                NW2 = HK // W2B
                for j in range(NW2 // 2):
                    emit_w2_quant(j)
                for i in range(NTL):
                    if i + 2 < NTL:
                        emit_X(i + 2)
                    emit_H(i)
                    if i == 0:
                        for j in range(NW2 // 2, NW2):
                            emit_w2_quant(j)
                    if i + 1 < NTL:
                        emit_FC1(i + 1)
                    emit_FC2(i)

    nc.compile()
    return nc


_BUILD_CACHE = {}


def _get_module(s_act, b_act, use_n1, use_n2, cfg=None):
    key = (s_act, b_act, use_n1, use_n2,
           tuple(sorted(cfg.items())) if cfg else None)
    if key not in _BUILD_CACHE:
        _BUILD_CACHE[key] = build(s_act, b_act, use_n1, use_n2, cfg=cfg)
    return _BUILD_CACHE[key]


def make_in_maps(x, norm1_w, w1, act_scale, act_bias, norm2_w, w2):
    xf = np.ascontiguousarray(np.asarray(x, np.float32).reshape(TOK_TOTAL, D))
    w1t = np.ascontiguousarray(np.asarray(w1, np.float32).T)   # [D, H]
    w2t = np.ascontiguousarray(np.asarray(w2, np.float32).T)   # [H, D]
    use_n1 = not np.all(norm1_w == 1.0)
    use_n2 = not np.all(norm2_w == 1.0)
    in_maps = []
    for c in range(N_CORES):
        m = {
            "x": xf[TOK * c:TOK * (c + 1)],
            "w1t": w1t,
            "w2t": w2t,
            "w1s": np.ascontiguousarray(w1t[(D // N_CORES) * c:(D // N_CORES) * (c + 1)]),
            "w2s": np.ascontiguousarray(w2t[(H // N_CORES) * c:(H // N_CORES) * (c + 1)]),
        }
        if use_n1:
            m["n1"] = np.asarray(norm1_w, np.float32).reshape(1, D)
        if use_n2:
            m["n2"] = np.asarray(norm2_w, np.float32).reshape(1, H)
        in_maps.append(m)
    return in_maps, use_n1, use_n2


def kernel(x, norm1_w, w1, act_scale, act_bias, norm2_w, w2):
    in_maps, use_n1, use_n2 = make_in_maps(
        x, norm1_w, w1, act_scale, act_bias, norm2_w, w2)
    s_act = float(np.asarray(act_scale).reshape(-1)[0])
    b_act = float(np.asarray(act_bias).reshape(-1)[0])
    nc = _get_module(s_act, b_act, use_n1, use_n2)
    res = run_bass_kernel_spmd(nc, in_maps, list(range(N_CORES)))
    y = np.concatenate([res.results[c]["y"] for c in range(N_CORES)], axis=0)
    return y.reshape(B, S, D).astype(np.float32)

